# revision 44
# baseline (speedup 1.0000x reference)
"""Trainium2 Bass kernel for CrossGraphAttention (gnn_message_passing).

Strategy (v4 — merged dual-branch chunks, shared canonical lane order):
  - Messages are linear in xt = W@x+b, so per-dst aggregation happens in
    20-dim x-space:  z_i = sum_j attn_ij * [x_j ; 1],  out_i = [W|b] @ z_i.
  - attn_ij = sigmoid(a_i[dst] + a_j[src] + ab) where a_x = ahat . x,
    ahat = W.T @ aW halves (computed on device from the weights).
  - dst nodes are sharded across the 8 cores (6250 each); every edge lives
    on its dst's core, so no all-reduce is needed.
  - ONE canonical lane order per core shared by BOTH branches (sorted by
    max(h_deg, k_deg)); under it the two branches' per-window column needs
    are nearly equal (+0.1%), so each compute chunk processes BOTH branches
    over the same window range with one uniform column count: every DVE /
    Pool / ACT op covers [h-block | k-block] at once, halving the op count
    (and its fixed per-op overhead + pipeline bubbles) vs per-branch chunks.
  - Host staging (index-structure only, no model arithmetic): windows of
    128 lanes; a DP packs windows into chunks padded to the chunk max C_w.
  - Per chunk, engines pipeline (emission is software-pipelined A_i, B_{i-1};
    xg is quad-buffered):
      DVE : prod = xe * ajr2 (bf16 2x, per-segment a_j row), t10, t5
      Pool: lg = 4-op sum of the 5 partials + per-window a_i bias from the
            concatenated aiwin2 (idle gpsimd; the final two chunks do this
            on DVE so Pool latency is not exposed at pipeline drain)
      ACT : attn2 = sigmoid(lg) duplicated pairs
      DVE : msg = xe * attn2 (pair-broadcast 2x); column-halving tree whose
            last level writes the concatenated zg2 directly
  - Output path per 512-lane group, paced window-ready THROUGHOUT the run
    (both branches complete windows together): 4 PE transposes into one
    PSUM tile + one ACT copy + [W|b] matmul per branch; gate logits via
    host-pre-broadcast lhsT gate weights (one accumulating PE matmul pair
    + one sigmoid, already replicated across partitions); 3 fusion DVE ops
    and the per-group y DMA trail by one chunk.
"""

import sys

sys.path.insert(0, "/opt/trn_rl_repo")

import numpy as np
import ml_dtypes

import concourse.bacc as bacc
import concourse.mybir as mybir
import concourse.tile as tile
from concourse.bass_utils import run_bass_kernel_spmd

F32 = mybir.dt.float32
BF16 = mybir.dt.bfloat16
AF = mybir.ActivationFunctionType
OP = mybir.AluOpType
BF = ml_dtypes.bfloat16

N_CORES = 8
C_IN = 19
C_OUT = 128
NF = 20            # features per slot row: [x(19) | flag]
SENTINEL = -60000.0
MAXCOLS = 192      # per-branch slot-columns per merged chunk (2x on device)
BRANCHES = ("h", "k")


# ----------------------------------------------------------------------------
# Host-side prep (index structure + data staging -- no model arithmetic)
# ----------------------------------------------------------------------------

def plan_chunks(cw, maxcols=MAXCOLS, head=2):
    """Chunks of consecutive windows, each padded to the chunk max C_w.
    DP minimizes slot-work + per-chunk overhead. `head` forces that many
    single-window leading chunks (small first DMAs so the DVE starts early).
    Returns list of (w0, nw, c)."""
    COLCYC, CHUNKCYC = 75.0, 1100.0
    chunks = [(w, 1, int(cw[w])) for w in range(head)]
    cw = [int(v) for v in cw]
    n = len(cw)
    INF = float("inf")
    best = [INF] * (n + 1)
    best[head] = 0.0
    back = [0] * (n + 1)
    for e in range(head + 1, n + 1):
        cmax = 0
        for s in range(e - 1, head - 1, -1):
            cmax = max(cmax, cw[s])
            nw = e - s
            if nw * cmax > maxcols:
                break
            cost = best[s] + nw * cmax * COLCYC + CHUNKCYC
            if cost < best[e]:
                best[e] = cost
                back[e] = s
    bounds = []
    e = n
    while e > head:
        s = back[e]
        bounds.append((s, e))
        e = s
    for (s, e) in reversed(bounds):
        chunks.append((s, e - s, max(cw[s:e])))
    return chunks


def host_prep(x, hyperedge_index, knn_edge_index):
    x = np.asarray(x, np.float32)
    N = x.shape[0]
    n_node = N // N_CORES
    n_lane = ((n_node + 127) // 128) * 128
    n_win = n_lane // 128
    xbf = x.astype(BF)

    edges = {"h": np.asarray(hyperedge_index), "k": np.asarray(knn_edge_index)}

    # per-(branch, core) local edge lists + canonical per-core order
    loc = {}
    orders, invs = [], []
    win_cw = {b: np.zeros((N_CORES, n_win), np.int64) for b in BRANCHES}
    for k in range(N_CORES):
        degs = {}
        for b in BRANCHES:
            src_all = edges[b][0].astype(np.int64)
            dst_all = edges[b][1].astype(np.int64)
            m = (dst_all // n_node) == k
            loc[(b, k)] = (src_all[m], dst_all[m] - k * n_node)
            degs[b] = np.bincount(loc[(b, k)][1], minlength=n_node)
        key = np.maximum(degs["h"], degs["k"])
        order = np.argsort(-key, kind="stable")
        inv = np.empty(n_node, np.int64)
        inv[order] = np.arange(n_node)
        orders.append(order)
        invs.append(inv)
        for b in BRANCHES:
            lane_deg = np.zeros(n_lane, np.int64)
            lane_deg[:n_node] = degs[b][order]
            win_cw[b][k] = lane_deg.reshape(n_win, 128).max(axis=1)

    # one shared window-width array (max over cores AND branches), DP-chunked
    cwm = np.maximum(win_cw["h"].max(axis=0), win_cw["k"].max(axis=0))
    plans = plan_chunks(cwm)
    cwp = np.zeros(n_win, np.int64)
    for (w0, nw, c) in plans:
        cwp[w0:w0 + nw] = c
    jw = np.concatenate([[0], np.cumsum(cwp)]).astype(np.int64)

    # per-core slot tables, both branches, then interleave per chunk
    in_maps = []
    for k in range(N_CORES):
        xe_b = {}
        for b in BRANCHES:
            n_cols = int(jw[-1])
            xe = np.zeros((128, n_cols, NF), BF)
            xe[:, :, C_IN] = BF(SENTINEL)
            src, dst_local = loc[(b, k)]
            lane = invs[k][dst_local]
            o = np.argsort(lane, kind="stable")
            lane_s = lane[o]
            src_s = src[o]
            first = np.searchsorted(lane_s, np.arange(n_lane))
            rank = np.arange(len(lane_s)) - first[lane_s]
            w = lane_s // 128
            p = lane_s % 128
            col = jw[w] + rank
            xe[p, col, :C_IN] = xbf[src_s]
            xe[p, col, C_IN] = BF(1.0)
            xe_b[b] = xe.reshape(128, n_cols * NF)
        blocks = []
        for (w0, nw, c) in plans:
            lo, hi = int(jw[w0]) * NF, int(jw[w0] + nw * c) * NF
            blocks.append(xe_b["h"][:, lo:hi])
            blocks.append(xe_b["k"][:, lo:hi])
        m = {"xe2": np.ascontiguousarray(np.concatenate(blocks, axis=1))}

        # dst [x|1] rows in canonical lane order: [128, n_win*20] bf16
        xd = np.zeros((128, n_win * NF), np.float32)
        lanes = np.arange(n_lane)
        real = lanes < n_node
        rows = np.zeros((n_lane, NF), np.float32)
        xc = x[k * n_node:(k + 1) * n_node]
        rows[real, :C_IN] = xc[orders[k][lanes[real]]]
        rows[:, C_IN] = 1.0
        xd[:, :] = rows.reshape(n_win, 128, NF).transpose(1, 0, 2).reshape(
            128, n_win * NF)
        m["xdst"] = xd.astype(BF)
        in_maps.append(m)

    meta = dict(N=N, n_node=n_node, n_lane=n_lane, orders=orders,
                cw=[int(v) for v in cwp], plans=plans)
    return meta, in_maps


def host_prep_weights(inputs):
    w = {}
    for b, pre in (("h", "hyper"), ("k", "knn")):
        W = np.asarray(inputs[f"{pre}_lin_W"], np.float32)
        bb = np.asarray(inputs[f"{pre}_lin_b"], np.float32).reshape(-1, 1)
        aW = np.asarray(inputs[f"{pre}_attn_W"], np.float32)
        ab = np.asarray(inputs[f"{pre}_attn_b"], np.float32)
        w[f"augW_{b}"] = np.ascontiguousarray(
            np.concatenate([W, bb], axis=1))                   # [128, 20]
        w[f"aWi_{b}"] = np.ascontiguousarray(aW[0, :C_OUT, None])
        w[f"aWj_{b}"] = np.ascontiguousarray(aW[0, C_OUT:, None])
        w[f"ab_{b}"] = ab.reshape(1, 1).astype(np.float32)
    gW = np.asarray(inputs["gate_W"], np.float32)
    gb = np.asarray(inputs["gate_b"], np.float32)
    # gate weights pre-broadcast along the free axis (pure layout): used as
    # lhsT so the gate logits come out replicated across all 128 partitions
    for row in (0, 1):
        for b, sl in (("h", slice(0, C_OUT)), ("k", slice(C_OUT, 2 * C_OUT))):
            w[f"gWbc_{b}{row}"] = np.ascontiguousarray(
                np.tile(gW[row, sl][:, None], (1, 128)).astype(BF))
        w[f"gbbc_{row}"] = np.full((128, 1), gb[row], np.float32)
    return w


# ----------------------------------------------------------------------------
# Device program
# ----------------------------------------------------------------------------

def build_program(meta):
    n_lane = meta["n_lane"]
    n_node = meta["n_node"]
    n_win = n_lane // 128

    nc = bacc.Bacc("TRN2", target_bir_lowering=False, debug=False,
                   num_devices=N_CORES)

    dram = {}

    def din(name, shape, dtype=F32):
        dram[name] = nc.dram_tensor(name, shape, dtype,
                                    kind="ExternalInput").ap()
        return dram[name]

    # chunk list: each device chunk = [h-block | k-block] of the window range
    chunks = []
    col0 = 0
    for (w0, nw, c) in meta["plans"]:
        chunks.append(dict(w0=w0, nw=nw, c=c, col0=col0,
                           ncols2=2 * nw * c))
        col0 += 2 * nw * c
    n_chunks = len(chunks)
    ncols_tot2 = col0

    din("xe2", [128, ncols_tot2 * NF], BF16)
    for b in BRANCHES:
        din(f"augW_{b}", [C_OUT, NF])
        din(f"aWi_{b}", [C_OUT, 1])
        din(f"aWj_{b}", [C_OUT, 1])
        din(f"ab_{b}", [1, 1])
    din("xdst", [128, n_win * NF], BF16)
    for row in (0, 1):
        for b in BRANCHES:
            din(f"gWbc_{b}{row}", [C_OUT, 128], BF16)
        din(f"gbbc_{row}", [128, 1])

    y = nc.dram_tensor("y", [C_OUT, n_node], BF16, kind="ExternalOutput").ap()

    import contextlib
    with tile.TileContext(nc) as tc, contextlib.ExitStack() as ctx:
        const = ctx.enter_context(tc.tile_pool(name="const", bufs=1))
        xpool = ctx.enter_context(tc.tile_pool(name="xpool", bufs=4))
        scr = ctx.enter_context(tc.tile_pool(name="scr", bufs=1))
        pp = ctx.enter_context(tc.tile_pool(name="pp", bufs=2))
        big = ctx.enter_context(tc.tile_pool(name="big", bufs=1))
        psum = ctx.enter_context(tc.tile_pool(name="psum", bufs=2,
                                              space="PSUM"))

        # --- xg prefetch machinery (first load emitted before everything) --
        xg_tiles = {}

        def load_xg(i):
            if i >= n_chunks or i in xg_tiles:
                return
            ch = chunks[i]
            t = xpool.tile([128, ch["ncols2"] * NF], BF16, tag="xg",
                           name="xg")
            nc.sync.dma_start(
                t[:], dram["xe2"][:, ch["col0"] * NF:
                                  (ch["col0"] + ch["ncols2"]) * NF])
            xg_tiles[i] = t

        load_xg(0)

        # --- identity / ones constants (gpsimd standard ops) ---------------
        ident = const.tile([128, 128], F32, tag="ident")
        nc.gpsimd.memset(ident[:], 0.0)
        nc.gpsimd.affine_select(
            out=ident[:], in_=ident[:], compare_op=OP.not_equal, fill=1.0,
            base=0, pattern=[[-1, 128]], channel_multiplier=1)
        ones1 = const.tile([1, 128], F32, tag="ones1")
        nc.gpsimd.memset(ones1[:], 1.0)
        identb = const.tile([128, 128], BF16, tag="identb")
        nc.vector.tensor_copy(identb[:], ident[:])

        # --- Phase A: weight-derived constants -----------------------------
        # ajr2 = [a_j row of h | a_j row of k] so one prod op serves both
        # segments of a merged chunk; aiwin2 likewise [aiwin_h | aiwin_k].
        augWT, air_bf = {}, {}
        ajr2 = const.tile([128, 2 * NF], BF16, tag="ajr2")
        aiwin2 = const.tile([128, 2 * n_win], F32, tag="aiwin2")
        SEG = {"h": 0, "k": 1}
        for b in ("k", "h"):
            wt = const.tile([C_OUT, NF], F32, tag=f"augW_{b}")
            nc.sync.dma_start(wt[:], dram[f"augW_{b}"][:])
            at = const.tile([1, 1], F32, tag=f"ab_{b}")
            nc.sync.dma_start(at[:], dram[f"ab_{b}"][:])
            raw = {}
            for side in ("i", "j"):
                av = const.tile([C_OUT, 1], F32, tag=f"aW{side}_{b}")
                nc.sync.dma_start(av[:], dram[f"aW{side}_{b}"][:])
                ps = psum.tile([128, NF], F32, tag="tps")
                nc.tensor.matmul(ps[0:1, :], lhsT=av[:], rhs=wt[:],
                                 start=True, stop=True)
                r = const.tile([1, NF], F32, tag=f"ahraw_{side}_{b}")
                nc.vector.tensor_copy(r[:], ps[0:1, :])   # [ahat | aW.b]
                raw[side] = r
            # total additive bias: aW_i.b + aW_j.b + ab - 1.0
            # (the -1.0 cancels the 1.0 flag channel through ajr col19)
            tot = const.tile([1, 1], F32, tag=f"tot_{b}")
            nc.vector.tensor_tensor(out=tot[:], in0=raw["i"][0:1, 19:20],
                                    in1=raw["j"][0:1, 19:20], op=OP.add)
            nc.vector.tensor_tensor(out=tot[:], in0=tot[:], in1=at[:],
                                    op=OP.add)
            nc.vector.tensor_scalar_add(tot[:], tot[:], -1.0)
            row_i = const.tile([1, NF], F32, tag=f"rowi_{b}")
            nc.vector.tensor_copy(row_i[:], raw["i"][:])
            nc.vector.tensor_copy(row_i[0:1, 19:20], tot[:])
            row_j = const.tile([1, NF], F32, tag=f"rowj_{b}")
            nc.vector.tensor_copy(row_j[:], raw["j"][:])
            nc.vector.memset(row_j[0:1, 19:20], 1.0)
            # replicate to 128 partitions
            psr = psum.tile([128, NF], F32, tag="tps")
            nc.tensor.matmul(psr[:], lhsT=ones1[:], rhs=row_i[:],
                             start=True, stop=True)
            air_t = const.tile([128, NF], BF16, tag=f"air_{b}")
            nc.vector.tensor_copy(air_t[:], psr[:])
            air_bf[b] = air_t
            psr2 = psum.tile([128, NF], F32, tag="tps")
            nc.tensor.matmul(psr2[:], lhsT=ones1[:], rhs=row_j[:],
                             start=True, stop=True)
            nc.vector.tensor_copy(
                ajr2[:, SEG[b] * NF:(SEG[b] + 1) * NF], psr2[:])

            # augWT = augW.T in bf16 for the output matmuls
            psA = psum.tile([NF, 128], F32, tag="tps")
            nc.tensor.transpose(psA[:], wt[:], ident[:])
            awt = const.tile([NF, 128], BF16, tag=f"augWT_{b}")
            nc.vector.tensor_copy(awt[:], psA[:])
            augWT[b] = awt

        xd = const.tile([128, n_win * NF], BF16, tag="xd", name="xd")
        nc.sync.dma_start(xd[:], dram["xdst"][:])
        gWbc, gbbc = {}, {}
        for row in (0, 1):
            for b in BRANCHES:
                t = const.tile([C_OUT, 128], BF16, tag=f"gWbc_{b}{row}")
                nc.sync.dma_start(t[:], dram[f"gWbc_{b}{row}"][:])
                gWbc[(b, row)] = t
            t = const.tile([128, 1], F32, tag=f"gbbc_{row}")
            nc.sync.dma_start(t[:], dram[f"gbbc_{row}"][:])
            gbbc[row] = t
        load_xg(1)
        load_xg(2)

        def emit_aiwin():
            # aiwin[p, w] = air . xdst  (DVE; overlaps the early xg DMAs)
            for b in BRANCHES:
                prodw = const.tile([128, n_win * NF], BF16, tag=f"xdp_{b}",
                                   name=f"xdp_{b}")
                nc.vector.tensor_tensor(
                    out=prodw[:].rearrange("p (w d) -> p w d", d=NF),
                    in0=xd[:].rearrange("p (w d) -> p w d", d=NF),
                    in1=air_bf[b][:, :].unsqueeze(1)
                        .broadcast_to([128, n_win, NF]),
                    op=OP.mult)
                nc.vector.tensor_reduce(
                    aiwin2[:, SEG[b] * n_win:(SEG[b] + 1) * n_win],
                    prodw[:].rearrange("p (w d) -> p w d", d=NF),
                    axis=mybir.AxisListType.X, op=OP.add)

        zg2 = big.tile([128, 2 * n_win * NF], BF16, tag="zg2", name="zg2")
        ot = {b: big.tile([128, n_lane], BF16, tag=f"ot_{b}",
                          name=f"ot_{b}")
              for b in BRANCHES}
        attn2s = {}

        # --- per-chunk stages (generators; driver interleaves DVE ops) -----
        def stage_A(i):
            ch = chunks[i]
            w0, nw, c, ncols2 = ch["w0"], ch["nw"], ch["c"], ch["ncols2"]
            xg = xg_tiles[i]
            prod = scr.tile([128, ncols2 * NF], BF16, tag="prod")
            nc.vector.tensor_tensor(
                out=prod[:].rearrange("p (s c f) -> p s c f", s=2, f=NF),
                in0=xg[:].rearrange("p (s c f) -> p s c f", s=2, f=NF),
                in1=ajr2[:].rearrange("p (s f) -> p s f", f=NF)
                    .unsqueeze(2).broadcast_to([128, 2, nw * c, NF]),
                op=OP.mult)
            yield
            p3 = prod[:].rearrange("p (c f) -> p c f", f=NF)
            t10 = scr.tile([128, ncols2 * 10], BF16, tag="t10")
            nc.vector.tensor_tensor(
                out=t10[:].rearrange("p (c f) -> p c f", f=10),
                in0=p3[:, :, 0:10], in1=p3[:, :, 10:20], op=OP.add)
            yield
            t103 = t10[:].rearrange("p (c f) -> p c f", f=10)
            t5 = pp.tile([128, ncols2 * 5], BF16, tag="t5")
            nc.vector.tensor_tensor(
                out=t5[:].rearrange("p (c f) -> p c f", f=5),
                in0=t103[:, :, 0:5], in1=t103[:, :, 5:10], op=OP.add)
            if i == 0:
                emit_aiwin()
            t53 = t5[:].rearrange("p (c f) -> p c f", f=5)
            lg = pp.tile([128, ncols2], F32, tag="lg")
            lgw = lg[:].rearrange("p (s w c) -> p s w c", s=2, c=c)
            aiw = (aiwin2[:].rearrange("p (s W) -> p s W", s=2)
                   [:, :, w0:w0 + nw].unsqueeze(3)
                   .broadcast_to([128, 2, nw, c]))
            if i >= n_chunks - 2 or i < 2:
                # head/tail chunks: logits on DVE -- the tiny ramp chunks
                # give the Pool+ACT chain no DVE work to hide under, and at
                # the drain there is no next chunk either
                nc.vector.tensor_reduce(lg[:], t53,
                                        axis=mybir.AxisListType.X,
                                        op=OP.add)
                nc.vector.tensor_tensor(out=lgw, in0=lgw, in1=aiw,
                                        op=OP.add)
            else:
                # Pool: logits = sum of the 5 partials + per-window bias
                t2 = pp.tile([128, ncols2 * 2], F32, tag="t2")
                t2v = t2[:].rearrange("p (c f) -> p c f", f=2)
                nc.gpsimd.tensor_tensor(out=t2v, in0=t53[:, :, 0:2],
                                        in1=t53[:, :, 2:4], op=OP.add)
                nc.gpsimd.tensor_tensor(out=lg[:], in0=t2v[:, :, 0],
                                        in1=t2v[:, :, 1], op=OP.add)
                nc.gpsimd.tensor_tensor(out=lg[:], in0=lg[:],
                                        in1=t53[:, :, 4], op=OP.add)
                nc.gpsimd.tensor_tensor(out=lgw, in0=lgw, in1=aiw,
                                        op=OP.add)
            # ACT: sigmoid into duplicated pairs
            attn2 = pp.tile([128, ncols2 * 2], BF16, tag="attn2")
            a3 = attn2[:].rearrange("p (c t) -> p c t", t=2)
            nc.scalar.activation(
                a3[:, :, 0:1].rearrange("p c o -> p (c o)"), lg[:],
                AF.Sigmoid)
            nc.scalar.activation(
                a3[:, :, 1:2].rearrange("p c o -> p (c o)"), lg[:],
                AF.Sigmoid)
            attn2s[i] = attn2

        def stage_B(i):
            ch = chunks[i]
            w0, nw, c, ncols2 = ch["w0"], ch["nw"], ch["c"], ch["ncols2"]
            xg = xg_tiles.pop(i)
            a3 = attn2s.pop(i)[:].rearrange("p (c t) -> p c t", t=2)
            msg = scr.tile([128, ncols2 * NF], BF16, tag="msg")
            nc.vector.tensor_tensor(
                out=msg[:].rearrange("p (c h t) -> p c h t", h=10, t=2),
                in0=xg[:].rearrange("p (c h t) -> p c h t", h=10, t=2),
                in1=a3[:, :, :].unsqueeze(2)
                    .broadcast_to([128, ncols2, 10, 2]),
                op=OP.mult)
            yield
            mv = msg[:].rearrange("p (W c f) -> p W c f", c=c, f=NF)
            zgv = (zg2[:].rearrange("p (s W f) -> p s W f", s=2, f=NF)
                   [:, :, w0:w0 + nw, :])
            cc = c
            while cc > 2:
                h = cc // 2
                nc.vector.tensor_tensor(
                    out=mv[:, :, 0:h, :], in0=mv[:, :, 0:h, :],
                    in1=mv[:, :, cc - h:cc, :], op=OP.add)
                yield
                cc -= h
            fin = mv[:, :, 0, :].rearrange("p (s w) f -> p s w f", s=2)
            if cc == 2:
                nc.vector.tensor_tensor(
                    out=zgv, in0=fin,
                    in1=mv[:, :, 1, :].rearrange("p (s w) f -> p s w f",
                                                 s=2),
                    op=OP.add)
            else:
                nc.vector.tensor_copy(zgv, fin)

        # --- per-group z transpose + [W|b] matmul (both branches) ----------
        def emit_zt_piece(b, g):
            w_lo, w_hi = 4 * g, min(4 * g + 4, n_win)
            n = (w_hi - w_lo) * 128
            sb = SEG[b] * n_win
            pst = psum.tile([NF, 512], BF16, tag="tps", name="pst")
            for w in range(w_lo, w_hi):
                nc.tensor.transpose(
                    pst[:, (w - w_lo) * 128:(w - w_lo) * 128 + 128],
                    zg2[:, (sb + w) * NF:(sb + w + 1) * NF], identb[:])
            zt = pp.tile([NF, 512], BF16, tag="zT", name="zt")
            nc.scalar.copy(zt[:, 0:n], pst[:, 0:n])
            pso = psum.tile([128, 512], F32, tag="pso", name="pso")
            nc.tensor.matmul(pso[:, 0:n], lhsT=augWT[b][:],
                             rhs=zt[:, 0:n], start=True, stop=True)
            nc.scalar.copy(ot[b][:, 512 * g:512 * g + n], pso[:, 0:n])

        # --- per-group gate chain (PE/ACT only) ----------------------------
        def emit_group_pe(g):
            w_lo, w_hi = 4 * g, min(4 * g + 4, n_win)
            n = (w_hi - w_lo) * 128
            c0 = 512 * g
            emit_zt_piece("k", g)
            emit_zt_piece("h", g)
            grs = []
            for row in (0, 1):
                psg = psum.tile([128, 512], F32, tag="psg", name="psg")
                nc.tensor.matmul(psg[:, 0:n], lhsT=gWbc[("h", row)][:],
                                 rhs=ot["h"][:, c0:c0 + n], start=True,
                                 stop=False)
                nc.tensor.matmul(psg[:, 0:n], lhsT=gWbc[("k", row)][:],
                                 rhs=ot["k"][:, c0:c0 + n], start=False,
                                 stop=True)
                grb = pp.tile([128, 512], BF16, tag=f"grb{row}",
                              name=f"grb{row}")
                nc.scalar.activation(grb[:, 0:n], psg[:, 0:n],
                                     AF.Sigmoid, bias=gbbc[row][:])
                grs.append(grb)
            return (c0, n, grs)

        def emit_group_fuse(st):
            c0, n, grs = st
            nc.vector.tensor_tensor(
                out=ot["h"][:, c0:c0 + n], in0=ot["h"][:, c0:c0 + n],
                in1=grs[0][:, 0:n], op=OP.mult)
            nc.vector.tensor_tensor(
                out=ot["k"][:, c0:c0 + n], in0=ot["k"][:, c0:c0 + n],
                in1=grs[1][:, 0:n], op=OP.mult)
            nc.vector.tensor_tensor(
                out=ot["h"][:, c0:c0 + n], in0=ot["h"][:, c0:c0 + n],
                in1=ot["k"][:, c0:c0 + n], op=OP.add)
            ny = min(c0 + n, n_node) - c0
            if ny > 0:
                nc.sync.dma_start(y[:, c0:c0 + ny], ot["h"][:, c0:c0 + ny])

        # --- main pipelined emission ---------------------------------------
        n_grp = (n_win + 3) // 4
        g_next = 0
        pending = []          # (state, chunk_idx_when_emitted)

        def after_B(j):
            nonlocal g_next
            # fusion with >= 1 chunk lag so the gate chain is done
            while pending and pending[0][1] < j:
                emit_group_fuse(pending.pop(0)[0])
            done_w = chunks[j]["w0"] + chunks[j]["nw"]
            while g_next < n_grp and (4 * (g_next + 1) <= done_w
                                      or done_w == n_win):
                pending.append((emit_group_pe(g_next), j))
                g_next += 1

        for i in range(n_chunks):
            load_xg(i + 2)
            load_xg(i + 3)
            active = [stage_B(i - 1)] if i > 0 else []
            active.append(stage_A(i))
            while active:
                nxt = []
                for g in active:
                    try:
                        next(g)
                        nxt.append(g)
                    except StopIteration:
                        pass
                active = nxt
            if i > 0:
                after_B(i - 1)
        for _ in stage_B(n_chunks - 1):
            pass
        after_B(n_chunks - 1)
        while pending:
            emit_group_fuse(pending.pop(0)[0])

    nc.compile()
    return nc


# ----------------------------------------------------------------------------
# Entry point
# ----------------------------------------------------------------------------

_CACHE = {}
LAST_EXEC_NS = None


def kernel(**inputs):
    x = np.asarray(inputs["x"], np.float32)
    N = x.shape[0]

    meta, in_maps = host_prep(x, inputs["hyperedge_index"],
                              inputs["knn_edge_index"])
    wmap = host_prep_weights(inputs)
    for m in in_maps:
        m.update(wmap)

    key = (meta["N"], tuple(meta["cw"]))
    if key not in _CACHE:
        _CACHE.clear()
        _CACHE[key] = build_program(meta)
    nc = _CACHE[key]

    import os
    global LAST_EXEC_NS
    trace = bool(int(os.environ.get("KERNEL_TRACE", "0")))
    res = run_bass_kernel_spmd(nc, in_maps, core_ids=list(range(N_CORES)),
                               trace=trace)
    LAST_EXEC_NS = res.exec_time_ns

    n_node = meta["n_node"]
    out = np.empty((N, C_OUT), np.float32)
    for k in range(N_CORES):
        yk = np.asarray(res.results[k]["y"]).astype(np.float32).T
        out[k * n_node + meta["orders"][k]] = yk
    return out


# revision 45
# speedup vs baseline: 1.0056x; 1.0056x over previous
"""Trainium2 Bass kernel for CrossGraphAttention (gnn_message_passing).

Strategy (v4 — merged dual-branch chunks, shared canonical lane order):
  - Messages are linear in xt = W@x+b, so per-dst aggregation happens in
    20-dim x-space:  z_i = sum_j attn_ij * [x_j ; 1],  out_i = [W|b] @ z_i.
  - attn_ij = sigmoid(a_i[dst] + a_j[src] + ab) where a_x = ahat . x,
    ahat = W.T @ aW halves (computed on device from the weights).
  - dst nodes are sharded across the 8 cores (6250 each); every edge lives
    on its dst's core, so no all-reduce is needed.
  - ONE canonical lane order per core shared by BOTH branches (sorted by
    max(h_deg, k_deg)); under it the two branches' per-window column needs
    are nearly equal (+0.1%), so each compute chunk processes BOTH branches
    over the same window range with one uniform column count: every DVE /
    Pool / ACT op covers [h-block | k-block] at once, halving the op count
    (and its fixed per-op overhead + pipeline bubbles) vs per-branch chunks.
  - Host staging (index-structure only, no model arithmetic): windows of
    128 lanes; a DP packs windows into chunks padded to the chunk max C_w.
  - Per chunk, engines pipeline (emission is software-pipelined A_i, B_{i-1};
    xg is quad-buffered):
      DVE : prod = xe * ajr2 (bf16 2x, per-segment a_j row), t10, t5
      Pool: lg = 4-op sum of the 5 partials + per-window a_i bias from the
            concatenated aiwin2 (idle gpsimd; the final two chunks do this
            on DVE so Pool latency is not exposed at pipeline drain)
      ACT : attn2 = sigmoid(lg) duplicated pairs
      DVE : msg = xe * attn2 (pair-broadcast 2x); column-halving tree whose
            last level writes the concatenated zg2 directly
  - Output path per 512-lane group, paced window-ready THROUGHOUT the run
    (both branches complete windows together): 4 PE transposes into one
    PSUM tile + one ACT copy + [W|b] matmul per branch; gate logits via
    host-pre-broadcast lhsT gate weights (one accumulating PE matmul pair
    + one sigmoid, already replicated across partitions); 3 fusion DVE ops
    and the per-group y DMA trail by one chunk.
"""

import sys

sys.path.insert(0, "/opt/trn_rl_repo")

import numpy as np
import ml_dtypes

import concourse.bacc as bacc
import concourse.mybir as mybir
import concourse.tile as tile
from concourse.bass_utils import run_bass_kernel_spmd

F32 = mybir.dt.float32
BF16 = mybir.dt.bfloat16
AF = mybir.ActivationFunctionType
OP = mybir.AluOpType
BF = ml_dtypes.bfloat16

N_CORES = 8
C_IN = 19
C_OUT = 128
NF = 20            # features per slot row: [x(19) | flag]
SENTINEL = -60000.0
MAXCOLS = 208      # per-branch slot-columns per merged chunk (2x on device)
BRANCHES = ("h", "k")


# ----------------------------------------------------------------------------
# Host-side prep (index structure + data staging -- no model arithmetic)
# ----------------------------------------------------------------------------

def plan_chunks(cw, maxcols=MAXCOLS, head=2):
    """Chunks of consecutive windows, each padded to the chunk max C_w.
    DP minimizes slot-work + per-chunk overhead. `head` forces that many
    single-window leading chunks (small first DMAs so the DVE starts early).
    Returns list of (w0, nw, c)."""
    COLCYC, CHUNKCYC = 75.0, 1100.0
    chunks = [(w, 1, int(cw[w])) for w in range(head)]
    cw = [int(v) for v in cw]
    n = len(cw)
    INF = float("inf")
    best = [INF] * (n + 1)
    best[head] = 0.0
    back = [0] * (n + 1)
    for e in range(head + 1, n + 1):
        cmax = 0
        for s in range(e - 1, head - 1, -1):
            cmax = max(cmax, cw[s])
            nw = e - s
            if nw * cmax > maxcols:
                break
            cost = best[s] + nw * cmax * COLCYC + CHUNKCYC
            if cost < best[e]:
                best[e] = cost
                back[e] = s
    bounds = []
    e = n
    while e > head:
        s = back[e]
        bounds.append((s, e))
        e = s
    for (s, e) in reversed(bounds):
        chunks.append((s, e - s, max(cw[s:e])))
    return chunks


def host_prep(x, hyperedge_index, knn_edge_index):
    x = np.asarray(x, np.float32)
    N = x.shape[0]
    n_node = N // N_CORES
    n_lane = ((n_node + 127) // 128) * 128
    n_win = n_lane // 128
    xbf = x.astype(BF)

    edges = {"h": np.asarray(hyperedge_index), "k": np.asarray(knn_edge_index)}

    # per-(branch, core) local edge lists + canonical per-core order
    loc = {}
    orders, invs = [], []
    win_cw = {b: np.zeros((N_CORES, n_win), np.int64) for b in BRANCHES}
    for k in range(N_CORES):
        degs = {}
        for b in BRANCHES:
            src_all = edges[b][0].astype(np.int64)
            dst_all = edges[b][1].astype(np.int64)
            m = (dst_all // n_node) == k
            loc[(b, k)] = (src_all[m], dst_all[m] - k * n_node)
            degs[b] = np.bincount(loc[(b, k)][1], minlength=n_node)
        key = np.maximum(degs["h"], degs["k"])
        order = np.argsort(-key, kind="stable")
        inv = np.empty(n_node, np.int64)
        inv[order] = np.arange(n_node)
        orders.append(order)
        invs.append(inv)
        for b in BRANCHES:
            lane_deg = np.zeros(n_lane, np.int64)
            lane_deg[:n_node] = degs[b][order]
            win_cw[b][k] = lane_deg.reshape(n_win, 128).max(axis=1)

    # one shared window-width array (max over cores AND branches), DP-chunked
    cwm = np.maximum(win_cw["h"].max(axis=0), win_cw["k"].max(axis=0))
    plans = plan_chunks(cwm)
    cwp = np.zeros(n_win, np.int64)
    for (w0, nw, c) in plans:
        cwp[w0:w0 + nw] = c
    jw = np.concatenate([[0], np.cumsum(cwp)]).astype(np.int64)

    # per-core slot tables, both branches, then interleave per chunk
    in_maps = []
    for k in range(N_CORES):
        xe_b = {}
        for b in BRANCHES:
            n_cols = int(jw[-1])
            xe = np.zeros((128, n_cols, NF), BF)
            xe[:, :, C_IN] = BF(SENTINEL)
            src, dst_local = loc[(b, k)]
            lane = invs[k][dst_local]
            o = np.argsort(lane, kind="stable")
            lane_s = lane[o]
            src_s = src[o]
            first = np.searchsorted(lane_s, np.arange(n_lane))
            rank = np.arange(len(lane_s)) - first[lane_s]
            w = lane_s // 128
            p = lane_s % 128
            col = jw[w] + rank
            xe[p, col, :C_IN] = xbf[src_s]
            xe[p, col, C_IN] = BF(1.0)
            xe_b[b] = xe.reshape(128, n_cols * NF)
        blocks = []
        for (w0, nw, c) in plans:
            lo, hi = int(jw[w0]) * NF, int(jw[w0] + nw * c) * NF
            blocks.append(xe_b["h"][:, lo:hi])
            blocks.append(xe_b["k"][:, lo:hi])
        m = {"xe2": np.ascontiguousarray(np.concatenate(blocks, axis=1))}

        # dst [x|1] rows in canonical lane order: [128, n_win*20] bf16
        xd = np.zeros((128, n_win * NF), np.float32)
        lanes = np.arange(n_lane)
        real = lanes < n_node
        rows = np.zeros((n_lane, NF), np.float32)
        xc = x[k * n_node:(k + 1) * n_node]
        rows[real, :C_IN] = xc[orders[k][lanes[real]]]
        rows[:, C_IN] = 1.0
        xd[:, :] = rows.reshape(n_win, 128, NF).transpose(1, 0, 2).reshape(
            128, n_win * NF)
        m["xdst"] = xd.astype(BF)
        in_maps.append(m)

    meta = dict(N=N, n_node=n_node, n_lane=n_lane, orders=orders,
                cw=[int(v) for v in cwp], plans=plans)
    return meta, in_maps


def host_prep_weights(inputs):
    w = {}
    for b, pre in (("h", "hyper"), ("k", "knn")):
        W = np.asarray(inputs[f"{pre}_lin_W"], np.float32)
        bb = np.asarray(inputs[f"{pre}_lin_b"], np.float32).reshape(-1, 1)
        aW = np.asarray(inputs[f"{pre}_attn_W"], np.float32)
        ab = np.asarray(inputs[f"{pre}_attn_b"], np.float32)
        w[f"augW_{b}"] = np.ascontiguousarray(
            np.concatenate([W, bb], axis=1))                   # [128, 20]
        w[f"aWi_{b}"] = np.ascontiguousarray(aW[0, :C_OUT, None])
        w[f"aWj_{b}"] = np.ascontiguousarray(aW[0, C_OUT:, None])
        w[f"ab_{b}"] = ab.reshape(1, 1).astype(np.float32)
    gW = np.asarray(inputs["gate_W"], np.float32)
    gb = np.asarray(inputs["gate_b"], np.float32)
    # gate weights pre-broadcast along the free axis (pure layout): used as
    # lhsT so the gate logits come out replicated across all 128 partitions
    for row in (0, 1):
        for b, sl in (("h", slice(0, C_OUT)), ("k", slice(C_OUT, 2 * C_OUT))):
            w[f"gWbc_{b}{row}"] = np.ascontiguousarray(
                np.tile(gW[row, sl][:, None], (1, 128)).astype(BF))
        w[f"gbbc_{row}"] = np.full((128, 1), gb[row], np.float32)
    return w


# ----------------------------------------------------------------------------
# Device program
# ----------------------------------------------------------------------------

def build_program(meta):
    n_lane = meta["n_lane"]
    n_node = meta["n_node"]
    n_win = n_lane // 128

    nc = bacc.Bacc("TRN2", target_bir_lowering=False, debug=False,
                   num_devices=N_CORES)

    dram = {}

    def din(name, shape, dtype=F32):
        dram[name] = nc.dram_tensor(name, shape, dtype,
                                    kind="ExternalInput").ap()
        return dram[name]

    # chunk list: each device chunk = [h-block | k-block] of the window range
    chunks = []
    col0 = 0
    for (w0, nw, c) in meta["plans"]:
        chunks.append(dict(w0=w0, nw=nw, c=c, col0=col0,
                           ncols2=2 * nw * c))
        col0 += 2 * nw * c
    n_chunks = len(chunks)
    ncols_tot2 = col0

    din("xe2", [128, ncols_tot2 * NF], BF16)
    for b in BRANCHES:
        din(f"augW_{b}", [C_OUT, NF])
        din(f"aWi_{b}", [C_OUT, 1])
        din(f"aWj_{b}", [C_OUT, 1])
        din(f"ab_{b}", [1, 1])
    din("xdst", [128, n_win * NF], BF16)
    for row in (0, 1):
        for b in BRANCHES:
            din(f"gWbc_{b}{row}", [C_OUT, 128], BF16)
        din(f"gbbc_{row}", [128, 1])

    y = nc.dram_tensor("y", [C_OUT, n_node], BF16, kind="ExternalOutput").ap()

    import contextlib
    with tile.TileContext(nc) as tc, contextlib.ExitStack() as ctx:
        const = ctx.enter_context(tc.tile_pool(name="const", bufs=1))
        xpool = ctx.enter_context(tc.tile_pool(name="xpool", bufs=4))
        scr = ctx.enter_context(tc.tile_pool(name="scr", bufs=1))
        pp = ctx.enter_context(tc.tile_pool(name="pp", bufs=2))
        big = ctx.enter_context(tc.tile_pool(name="big", bufs=1))
        psum = ctx.enter_context(tc.tile_pool(name="psum", bufs=2,
                                              space="PSUM"))

        # --- xg prefetch machinery (first load emitted before everything) --
        xg_tiles = {}

        def load_xg(i):
            if i >= n_chunks or i in xg_tiles:
                return
            ch = chunks[i]
            t = xpool.tile([128, ch["ncols2"] * NF], BF16, tag="xg",
                           name="xg")
            nc.sync.dma_start(
                t[:], dram["xe2"][:, ch["col0"] * NF:
                                  (ch["col0"] + ch["ncols2"]) * NF])
            xg_tiles[i] = t

        load_xg(0)

        # --- identity / ones constants (gpsimd standard ops) ---------------
        ident = const.tile([128, 128], F32, tag="ident")
        nc.gpsimd.memset(ident[:], 0.0)
        nc.gpsimd.affine_select(
            out=ident[:], in_=ident[:], compare_op=OP.not_equal, fill=1.0,
            base=0, pattern=[[-1, 128]], channel_multiplier=1)
        ones1 = const.tile([1, 128], F32, tag="ones1")
        nc.gpsimd.memset(ones1[:], 1.0)
        identb = const.tile([128, 128], BF16, tag="identb")
        nc.vector.tensor_copy(identb[:], ident[:])

        # --- Phase A: weight-derived constants -----------------------------
        # ajr2 = [a_j row of h | a_j row of k] so one prod op serves both
        # segments of a merged chunk; aiwin2 likewise [aiwin_h | aiwin_k].
        augWT, air_bf = {}, {}
        ajr2 = const.tile([128, 2 * NF], BF16, tag="ajr2")
        aiwin2 = const.tile([128, 2 * n_win], F32, tag="aiwin2")
        SEG = {"h": 0, "k": 1}
        for b in ("k", "h"):
            wt = const.tile([C_OUT, NF], F32, tag=f"augW_{b}")
            nc.sync.dma_start(wt[:], dram[f"augW_{b}"][:])
            at = const.tile([1, 1], F32, tag=f"ab_{b}")
            nc.sync.dma_start(at[:], dram[f"ab_{b}"][:])
            raw = {}
            for side in ("i", "j"):
                av = const.tile([C_OUT, 1], F32, tag=f"aW{side}_{b}")
                nc.sync.dma_start(av[:], dram[f"aW{side}_{b}"][:])
                ps = psum.tile([128, NF], F32, tag="tps")
                nc.tensor.matmul(ps[0:1, :], lhsT=av[:], rhs=wt[:],
                                 start=True, stop=True)
                r = const.tile([1, NF], F32, tag=f"ahraw_{side}_{b}")
                nc.vector.tensor_copy(r[:], ps[0:1, :])   # [ahat | aW.b]
                raw[side] = r
            # total additive bias: aW_i.b + aW_j.b + ab - 1.0
            # (the -1.0 cancels the 1.0 flag channel through ajr col19)
            tot = const.tile([1, 1], F32, tag=f"tot_{b}")
            nc.vector.tensor_tensor(out=tot[:], in0=raw["i"][0:1, 19:20],
                                    in1=raw["j"][0:1, 19:20], op=OP.add)
            nc.vector.tensor_tensor(out=tot[:], in0=tot[:], in1=at[:],
                                    op=OP.add)
            nc.vector.tensor_scalar_add(tot[:], tot[:], -1.0)
            row_i = const.tile([1, NF], F32, tag=f"rowi_{b}")
            nc.vector.tensor_copy(row_i[:], raw["i"][:])
            nc.vector.tensor_copy(row_i[0:1, 19:20], tot[:])
            row_j = const.tile([1, NF], F32, tag=f"rowj_{b}")
            nc.vector.tensor_copy(row_j[:], raw["j"][:])
            nc.vector.memset(row_j[0:1, 19:20], 1.0)
            # replicate to 128 partitions
            psr = psum.tile([128, NF], F32, tag="tps")
            nc.tensor.matmul(psr[:], lhsT=ones1[:], rhs=row_i[:],
                             start=True, stop=True)
            air_t = const.tile([128, NF], BF16, tag=f"air_{b}")
            nc.vector.tensor_copy(air_t[:], psr[:])
            air_bf[b] = air_t
            psr2 = psum.tile([128, NF], F32, tag="tps")
            nc.tensor.matmul(psr2[:], lhsT=ones1[:], rhs=row_j[:],
                             start=True, stop=True)
            nc.vector.tensor_copy(
                ajr2[:, SEG[b] * NF:(SEG[b] + 1) * NF], psr2[:])

            # augWT = augW.T in bf16 for the output matmuls
            psA = psum.tile([NF, 128], F32, tag="tps")
            nc.tensor.transpose(psA[:], wt[:], ident[:])
            awt = const.tile([NF, 128], BF16, tag=f"augWT_{b}")
            nc.vector.tensor_copy(awt[:], psA[:])
            augWT[b] = awt

        xd = const.tile([128, n_win * NF], BF16, tag="xd", name="xd")
        nc.sync.dma_start(xd[:], dram["xdst"][:])
        gWbc, gbbc = {}, {}
        for row in (0, 1):
            for b in BRANCHES:
                t = const.tile([C_OUT, 128], BF16, tag=f"gWbc_{b}{row}")
                nc.sync.dma_start(t[:], dram[f"gWbc_{b}{row}"][:])
                gWbc[(b, row)] = t
            t = const.tile([128, 1], F32, tag=f"gbbc_{row}")
            nc.sync.dma_start(t[:], dram[f"gbbc_{row}"][:])
            gbbc[row] = t
        load_xg(1)
        load_xg(2)

        def emit_aiwin():
            # aiwin[p, w] = air . xdst  (DVE; overlaps the early xg DMAs)
            for b in BRANCHES:
                prodw = const.tile([128, n_win * NF], BF16, tag=f"xdp_{b}",
                                   name=f"xdp_{b}")
                nc.vector.tensor_tensor(
                    out=prodw[:].rearrange("p (w d) -> p w d", d=NF),
                    in0=xd[:].rearrange("p (w d) -> p w d", d=NF),
                    in1=air_bf[b][:, :].unsqueeze(1)
                        .broadcast_to([128, n_win, NF]),
                    op=OP.mult)
                nc.vector.tensor_reduce(
                    aiwin2[:, SEG[b] * n_win:(SEG[b] + 1) * n_win],
                    prodw[:].rearrange("p (w d) -> p w d", d=NF),
                    axis=mybir.AxisListType.X, op=OP.add)

        zg2 = big.tile([128, 2 * n_win * NF], BF16, tag="zg2", name="zg2")
        ot = {b: big.tile([128, n_lane], BF16, tag=f"ot_{b}",
                          name=f"ot_{b}")
              for b in BRANCHES}
        attn2s = {}

        # --- per-chunk stages (generators; driver interleaves DVE ops) -----
        def stage_A(i):
            ch = chunks[i]
            w0, nw, c, ncols2 = ch["w0"], ch["nw"], ch["c"], ch["ncols2"]
            xg = xg_tiles[i]
            prod = scr.tile([128, ncols2 * NF], BF16, tag="prod")
            nc.vector.tensor_tensor(
                out=prod[:].rearrange("p (s c f) -> p s c f", s=2, f=NF),
                in0=xg[:].rearrange("p (s c f) -> p s c f", s=2, f=NF),
                in1=ajr2[:].rearrange("p (s f) -> p s f", f=NF)
                    .unsqueeze(2).broadcast_to([128, 2, nw * c, NF]),
                op=OP.mult)
            yield
            p3 = prod[:].rearrange("p (c f) -> p c f", f=NF)
            t10 = scr.tile([128, ncols2 * 10], BF16, tag="t10")
            nc.vector.tensor_tensor(
                out=t10[:].rearrange("p (c f) -> p c f", f=10),
                in0=p3[:, :, 0:10], in1=p3[:, :, 10:20], op=OP.add)
            yield
            t103 = t10[:].rearrange("p (c f) -> p c f", f=10)
            t5 = pp.tile([128, ncols2 * 5], BF16, tag="t5")
            nc.vector.tensor_tensor(
                out=t5[:].rearrange("p (c f) -> p c f", f=5),
                in0=t103[:, :, 0:5], in1=t103[:, :, 5:10], op=OP.add)
            if i == 0:
                emit_aiwin()
            t53 = t5[:].rearrange("p (c f) -> p c f", f=5)
            lg = pp.tile([128, ncols2], F32, tag="lg")
            lgw = lg[:].rearrange("p (s w c) -> p s w c", s=2, c=c)
            aiw = (aiwin2[:].rearrange("p (s W) -> p s W", s=2)
                   [:, :, w0:w0 + nw].unsqueeze(3)
                   .broadcast_to([128, 2, nw, c]))
            if i >= n_chunks - 2 or i < 2:
                # head/tail chunks: logits on DVE -- the tiny ramp chunks
                # give the Pool+ACT chain no DVE work to hide under, and at
                # the drain there is no next chunk either
                nc.vector.tensor_reduce(lg[:], t53,
                                        axis=mybir.AxisListType.X,
                                        op=OP.add)
                nc.vector.tensor_tensor(out=lgw, in0=lgw, in1=aiw,
                                        op=OP.add)
            else:
                # Pool: logits = sum of the 5 partials + per-window bias
                t2 = pp.tile([128, ncols2 * 2], F32, tag="t2")
                t2v = t2[:].rearrange("p (c f) -> p c f", f=2)
                nc.gpsimd.tensor_tensor(out=t2v, in0=t53[:, :, 0:2],
                                        in1=t53[:, :, 2:4], op=OP.add)
                nc.gpsimd.tensor_tensor(out=lg[:], in0=t2v[:, :, 0],
                                        in1=t2v[:, :, 1], op=OP.add)
                nc.gpsimd.tensor_tensor(out=lg[:], in0=lg[:],
                                        in1=t53[:, :, 4], op=OP.add)
                nc.gpsimd.tensor_tensor(out=lgw, in0=lgw, in1=aiw,
                                        op=OP.add)
            # ACT: sigmoid into duplicated pairs
            attn2 = pp.tile([128, ncols2 * 2], BF16, tag="attn2")
            a3 = attn2[:].rearrange("p (c t) -> p c t", t=2)
            nc.scalar.activation(
                a3[:, :, 0:1].rearrange("p c o -> p (c o)"), lg[:],
                AF.Sigmoid)
            nc.scalar.activation(
                a3[:, :, 1:2].rearrange("p c o -> p (c o)"), lg[:],
                AF.Sigmoid)
            attn2s[i] = attn2

        def stage_B(i):
            ch = chunks[i]
            w0, nw, c, ncols2 = ch["w0"], ch["nw"], ch["c"], ch["ncols2"]
            xg = xg_tiles.pop(i)
            a3 = attn2s.pop(i)[:].rearrange("p (c t) -> p c t", t=2)
            msg = scr.tile([128, ncols2 * NF], BF16, tag="msg")
            nc.vector.tensor_tensor(
                out=msg[:].rearrange("p (c h t) -> p c h t", h=10, t=2),
                in0=xg[:].rearrange("p (c h t) -> p c h t", h=10, t=2),
                in1=a3[:, :, :].unsqueeze(2)
                    .broadcast_to([128, ncols2, 10, 2]),
                op=OP.mult)
            yield
            mv = msg[:].rearrange("p (W c f) -> p W c f", c=c, f=NF)
            zgv = (zg2[:].rearrange("p (s W f) -> p s W f", s=2, f=NF)
                   [:, :, w0:w0 + nw, :])
            cc = c
            while cc > 2:
                h = cc // 2
                nc.vector.tensor_tensor(
                    out=mv[:, :, 0:h, :], in0=mv[:, :, 0:h, :],
                    in1=mv[:, :, cc - h:cc, :], op=OP.add)
                yield
                cc -= h
            fin = mv[:, :, 0, :].rearrange("p (s w) f -> p s w f", s=2)
            if cc == 2:
                nc.vector.tensor_tensor(
                    out=zgv, in0=fin,
                    in1=mv[:, :, 1, :].rearrange("p (s w) f -> p s w f",
                                                 s=2),
                    op=OP.add)
            else:
                nc.vector.tensor_copy(zgv, fin)

        # --- per-group z transpose + [W|b] matmul (both branches) ----------
        def emit_zt_piece(b, g):
            w_lo, w_hi = 4 * g, min(4 * g + 4, n_win)
            n = (w_hi - w_lo) * 128
            sb = SEG[b] * n_win
            pst = psum.tile([NF, 512], BF16, tag="tps", name="pst")
            for w in range(w_lo, w_hi):
                nc.tensor.transpose(
                    pst[:, (w - w_lo) * 128:(w - w_lo) * 128 + 128],
                    zg2[:, (sb + w) * NF:(sb + w + 1) * NF], identb[:])
            zt = pp.tile([NF, 512], BF16, tag="zT", name="zt")
            nc.scalar.copy(zt[:, 0:n], pst[:, 0:n])
            pso = psum.tile([128, 512], F32, tag="pso", name="pso")
            nc.tensor.matmul(pso[:, 0:n], lhsT=augWT[b][:],
                             rhs=zt[:, 0:n], start=True, stop=True)
            nc.scalar.copy(ot[b][:, 512 * g:512 * g + n], pso[:, 0:n])

        # --- per-group gate chain (PE/ACT only) ----------------------------
        def emit_group_pe(g):
            w_lo, w_hi = 4 * g, min(4 * g + 4, n_win)
            n = (w_hi - w_lo) * 128
            c0 = 512 * g
            emit_zt_piece("k", g)
            emit_zt_piece("h", g)
            grs = []
            for row in (0, 1):
                psg = psum.tile([128, 512], F32, tag="psg", name="psg")
                nc.tensor.matmul(psg[:, 0:n], lhsT=gWbc[("h", row)][:],
                                 rhs=ot["h"][:, c0:c0 + n], start=True,
                                 stop=False)
                nc.tensor.matmul(psg[:, 0:n], lhsT=gWbc[("k", row)][:],
                                 rhs=ot["k"][:, c0:c0 + n], start=False,
                                 stop=True)
                grb = pp.tile([128, 512], BF16, tag=f"grb{row}",
                              name=f"grb{row}")
                nc.scalar.activation(grb[:, 0:n], psg[:, 0:n],
                                     AF.Sigmoid, bias=gbbc[row][:])
                grs.append(grb)
            return (c0, n, grs)

        def emit_group_fuse(st):
            c0, n, grs = st
            nc.vector.tensor_tensor(
                out=ot["h"][:, c0:c0 + n], in0=ot["h"][:, c0:c0 + n],
                in1=grs[0][:, 0:n], op=OP.mult)
            nc.vector.tensor_tensor(
                out=ot["k"][:, c0:c0 + n], in0=ot["k"][:, c0:c0 + n],
                in1=grs[1][:, 0:n], op=OP.mult)
            nc.vector.tensor_tensor(
                out=ot["h"][:, c0:c0 + n], in0=ot["h"][:, c0:c0 + n],
                in1=ot["k"][:, c0:c0 + n], op=OP.add)
            ny = min(c0 + n, n_node) - c0
            if ny > 0:
                nc.sync.dma_start(y[:, c0:c0 + ny], ot["h"][:, c0:c0 + ny])

        # --- main pipelined emission ---------------------------------------
        n_grp = (n_win + 3) // 4
        g_next = 0
        pending = []          # (state, chunk_idx_when_emitted)

        def after_B(j):
            nonlocal g_next
            # fusion with >= 1 chunk lag so the gate chain is done
            while pending and pending[0][1] < j:
                emit_group_fuse(pending.pop(0)[0])
            done_w = chunks[j]["w0"] + chunks[j]["nw"]
            while g_next < n_grp and (4 * (g_next + 1) <= done_w
                                      or done_w == n_win):
                pending.append((emit_group_pe(g_next), j))
                g_next += 1

        for i in range(n_chunks):
            load_xg(i + 2)
            load_xg(i + 3)
            active = [stage_B(i - 1)] if i > 0 else []
            active.append(stage_A(i))
            while active:
                nxt = []
                for g in active:
                    try:
                        next(g)
                        nxt.append(g)
                    except StopIteration:
                        pass
                active = nxt
            if i > 0:
                after_B(i - 1)
        for _ in stage_B(n_chunks - 1):
            pass
        after_B(n_chunks - 1)
        while pending:
            emit_group_fuse(pending.pop(0)[0])

    nc.compile()
    return nc


# ----------------------------------------------------------------------------
# Entry point
# ----------------------------------------------------------------------------

_CACHE = {}
LAST_EXEC_NS = None


def kernel(**inputs):
    x = np.asarray(inputs["x"], np.float32)
    N = x.shape[0]

    meta, in_maps = host_prep(x, inputs["hyperedge_index"],
                              inputs["knn_edge_index"])
    wmap = host_prep_weights(inputs)
    for m in in_maps:
        m.update(wmap)

    key = (meta["N"], tuple(meta["cw"]))
    if key not in _CACHE:
        _CACHE.clear()
        _CACHE[key] = build_program(meta)
    nc = _CACHE[key]

    import os
    global LAST_EXEC_NS
    trace = bool(int(os.environ.get("KERNEL_TRACE", "0")))
    res = run_bass_kernel_spmd(nc, in_maps, core_ids=list(range(N_CORES)),
                               trace=trace)
    LAST_EXEC_NS = res.exec_time_ns

    n_node = meta["n_node"]
    out = np.empty((N, C_OUT), np.float32)
    for k in range(N_CORES):
        yk = np.asarray(res.results[k]["y"]).astype(np.float32).T
        out[k * n_node + meta["orders"][k]] = yk
    return out
